# revision 10
# baseline (speedup 1.0000x reference)
"""Trainium2 Bass kernel for the GatedODEFlow problem.

Math: the reference iterates  a <- a + h*alpha(a) * (tgt - a)  where
alpha depends on a only through the low-rank projection (a - mu) @ U / S.
Since each step is a per-row convex blend toward the fixed vector tgt,
a_t = c_t * x + (1 - c_t) * tgt  for a per-row scalar c_t, and the
projection evolves affinely in c_t:

    proj_t = c_t * (x@W - tgt@W) + (tgt@W - mu@W)   with W = U / (S+1e-6)
    dist2_t = A * c_t^2 + B2 * c_t + C              (per-row A, B2; global C)
    alpha_t = exp(-dist2_t / (2*k*sigma^2))
    c_{t+1} = c_t * (1 - h * alpha_t),  c_0 = 1
    out = c_N * x + (1 - c_N) * tgt

So the device only needs ONE matmul q0 = x @ W per row plus a scalar
recurrence and a final fused blend: read x once, write out once
(memory-bound roofline).

The gate path (projection -> per-row scalars) runs in bf16 on the tensor
engine: it only determines the scalar alpha, whose sensitivity to input
rounding is tiny (dist^2 ~ hundreds, /128 in the exponent). The final
blend out = c*x + (1-c)*tgt reads the original fp32 x, so output
precision stays ~1e-4.

Sharding: data-parallel across 8 cores along the batch dim; small
parameters replicated (per the problem's sharding hint).
"""

import math
import os
from contextlib import ExitStack

import numpy as np
import ml_dtypes

import concourse.bass as bass
import concourse.mybir as mybir
import concourse.tile as tile
from concourse import bacc
from concourse.masks import make_identity
from concourse.bass_utils import run_bass_kernel_spmd

F32 = mybir.dt.float32
BF16 = mybir.dt.bfloat16
AF = mybir.ActivationFunctionType
OP = mybir.AluOpType

N_CORES = 8
D = 4096
KSUB = 64
SUB = 128            # rows per subblock (one partition tile)
SPM = 4              # subblocks per macroblock
MACRO = SUB * SPM    # 512 rows
DCH = 128            # d-chunk width for PE transposes
NDCH = D // DCH      # 32
XCH = 1024           # bf16 cast chunk width
NXCH = D // XCH      # 4
CCH = 512            # combine chunk width
NCCH = D // CCH      # 8

_PROGRAM_CACHE: dict = {}
LAST_RESULT = None


def _build_program(rows: int, num_steps: int, neg_inv: float, exp_bias: float,
                   neg_h: float):
    nmacro = rows // MACRO
    assert rows == nmacro * MACRO, f"rows {rows} not a multiple of {MACRO}"

    nc = bacc.Bacc("TRN2")
    x_d = nc.dram_tensor("x", [rows, D], F32, kind="ExternalInput")
    w_d = nc.dram_tensor("w", [D, KSUB], BF16, kind="ExternalInput")
    tgt_d = nc.dram_tensor("tgt", [1, D], BF16, kind="ExternalInput")
    nqt_d = nc.dram_tensor("nqt", [KSUB, 1], F32, kind="ExternalInput")
    abr_d = nc.dram_tensor("abr", [128, 2], F32, kind="ExternalInput")
    out_d = nc.dram_tensor("out", [rows, D], F32, kind="ExternalOutput")

    with ExitStack() as ctx:
        tc = ctx.enter_context(tile.TileContext(nc))
        singles = ctx.enter_context(tc.tile_pool(name="singles", bufs=1))
        xpool = ctx.enter_context(tc.tile_pool(name="xp", bufs=8))
        x16pool = ctx.enter_context(tc.tile_pool(name="x16p", bufs=10))
        xtpool = ctx.enter_context(tc.tile_pool(name="xtp", bufs=3))
        stkpool = ctx.enter_context(tc.tile_pool(name="stkp", bufs=2))
        smpool = ctx.enter_context(tc.tile_pool(name="smp", bufs=2))
        ptr = ctx.enter_context(tc.tile_pool(name="ptr", bufs=2, space="PSUM"))
        pq = ctx.enter_context(tc.tile_pool(name="pq", bufs=1, space="PSUM"))
        pab = ctx.enter_context(tc.tile_pool(name="pab", bufs=1, space="PSUM"))
        pdt = ctx.enter_context(tc.tile_pool(name="pdt", bufs=1, space="PSUM"))
        pout = ctx.enter_context(tc.tile_pool(name="pout", bufs=3, space="PSUM"))

        ident16 = singles.tile([128, 128], BF16)
        make_identity(nc, ident16)
        ident32 = singles.tile([128, 128], F32)
        make_identity(nc, ident32)
        w_sb = singles.tile([128, NDCH, KSUB], BF16)
        nc.sync.dma_start(out=w_sb, in_=w_d[:, :].rearrange("(j p) k -> p j k", p=128))
        tgt_sb = singles.tile([1, D], BF16)
        nc.sync.dma_start(out=tgt_sb, in_=tgt_d[:, :])
        nqt_sb = singles.tile([KSUB, 1], F32)
        nc.sync.dma_start(out=nqt_sb, in_=nqt_d[:, :])
        abr_sb = singles.tile([128, 2], F32)
        nc.sync.dma_start(out=abr_sb, in_=abr_d[:, :])
        ebias_sb = singles.tile([128, 1], F32)
        nc.vector.memset(ebias_sb, exp_bias)

        for m in range(nmacro):
            r0 = m * MACRO
            # -------- load x tiles (natural layout, kept for the blend) ----
            xs = []
            for s in range(SPM):
                xin = xpool.tile([SUB, D], F32, tag="xin")
                nc.sync.dma_start(
                    out=xin, in_=x_d[r0 + s * SUB : r0 + (s + 1) * SUB, :])
                xs.append(xin)

            # -------- bf16 copy of x for the gate path (GPSIMD, idle) ------
            x16 = [[None] * NXCH for _ in range(SPM)]
            for cc in range(NXCH):
                for s in range(SPM):
                    t16 = x16pool.tile([SUB, XCH], BF16, tag="x16")
                    nc.gpsimd.tensor_copy(
                        t16, xs[s][:, cc * XCH : (cc + 1) * XCH])
                    x16[s][cc] = t16

            # -------- projection q0T[k, b] = W.T @ x.T  (bf16) -------------
            q0T = pq.tile([KSUB, MACRO], F32, tag="q0T")
            for j in range(NDCH):
                cc, jj = divmod(j, XCH // DCH)
                tp = ptr.tile([128, MACRO], BF16, tag="tp")
                for s in range(SPM):
                    nc.tensor.transpose(
                        tp[:, s * SUB : (s + 1) * SUB],
                        x16[s][cc][:, jj * DCH : (jj + 1) * DCH], ident16)
                xt = xtpool.tile([128, MACRO], BF16, tag="xt")
                nc.scalar.copy(xt, tp)
                nc.tensor.matmul(
                    q0T, w_sb[:, j, :], xt,
                    start=(j == 0), stop=(j == NDCH - 1))

            # -------- per-row A and B2 via tiny reductions -----------------
            # stk rows 0..63  = (q0T - qT)^2 ; rows 64..127 = (q0T - qT)
            stk = stkpool.tile([128, MACRO], F32, tag="stk")
            nc.scalar.activation(stk[0:KSUB, :], q0T, AF.Square,
                                 bias=nqt_sb, scale=1.0)
            nc.scalar.activation(stk[KSUB:128, :], q0T, AF.Identity,
                                 bias=nqt_sb, scale=1.0)
            ab = pab.tile([128, 2 * SPM], F32, tag="ab")
            for s in range(SPM):
                lhs = stk[:, s * SUB : (s + 1) * SUB]
                nc.tensor.matmul(ab[:, s : s + 1], lhs,
                                 abr_sb[:, 0:1],
                                 start=True, stop=True)
                nc.tensor.matmul(ab[:, SPM + s : SPM + s + 1], lhs,
                                 abr_sb[:, 1:2],
                                 start=True, stop=True)
            A = ab[:, 0:SPM]
            B2 = ab[:, SPM : 2 * SPM]

            # -------- scalar recurrence on c  [128, SPM] -------------------
            c = smpool.tile([128, SPM], F32, tag="c")
            nc.vector.memset(c, 1.0)
            t1 = smpool.tile([128, SPM], F32, tag="t1")
            alpha = smpool.tile([128, SPM], F32, tag="alpha")
            for _t in range(num_steps):
                nc.vector.tensor_tensor(t1, A, c, OP.mult)
                nc.vector.tensor_tensor(t1, t1, B2, OP.add)
                nc.vector.tensor_tensor(t1, t1, c, OP.mult)
                # alpha = exp(neg_inv * dist' + (-inv*C))
                nc.scalar.activation(alpha, t1, AF.Exp,
                                     bias=ebias_sb, scale=neg_inv)
                nc.vector.tensor_tensor(t1, alpha, c, OP.mult)
                # c = (t1 * -h) + c
                nc.vector.scalar_tensor_tensor(c, t1, neg_h, c, OP.mult, OP.add)

            # d = 1 - c, transposed into a free-dim row per subblock
            d_t = smpool.tile([128, SPM], F32, tag="d")
            nc.vector.tensor_scalar(d_t, c, -1.0, 1.0, OP.mult, OP.add)
            dT = pdt.tile([1, MACRO], F32, tag="dT")
            drows = []
            for s in range(SPM):
                nc.tensor.transpose(dT[:, s * SUB : (s + 1) * SUB],
                                    d_t[:, s : s + 1], ident32)
                dr = smpool.tile([1, SUB], BF16, tag=f"dr{s}")
                nc.vector.tensor_copy(dr, dT[:, s * SUB : (s + 1) * SUB])
                drows.append(dr)

            # -------- blend: out = c*x + (1-c)*tgt, in place over x --------
            for s in range(SPM):
                for h2 in range(NCCH):
                    op_ps = pout.tile([128, CCH], F32, tag="op")
                    nc.tensor.matmul(
                        op_ps, drows[s],
                        tgt_sb[:, h2 * CCH : (h2 + 1) * CCH],
                        start=True, stop=True)
                    xsl = xs[s][:, h2 * CCH : (h2 + 1) * CCH]
                    nc.vector.scalar_tensor_tensor(
                        xsl, xsl, c[:, s : s + 1], op_ps, OP.mult, OP.add)
                nc.sync.dma_start(
                    out=out_d[r0 + s * SUB : r0 + (s + 1) * SUB, :], in_=xs[s])

    if not nc.is_finalized():
        nc.finalize()
    return nc


def _get_program(rows, num_steps, neg_inv, exp_bias, neg_h):
    key = (rows, num_steps, neg_inv, exp_bias, neg_h)
    if key not in _PROGRAM_CACHE:
        _PROGRAM_CACHE[key] = _build_program(rows, num_steps, neg_inv,
                                             exp_bias, neg_h)
    return _PROGRAM_CACHE[key]


def kernel(x, manifold_mu, manifold_U, manifold_S, attractor_mu,
           log_step, sigma, num_steps):
    global LAST_RESULT
    x = np.ascontiguousarray(np.asarray(x, dtype=np.float32))
    mu = np.asarray(manifold_mu, dtype=np.float64)
    U = np.asarray(manifold_U, dtype=np.float64)
    S = np.asarray(manifold_S, dtype=np.float64)
    tgt = np.asarray(attractor_mu, dtype=np.float64)
    ls = float(np.asarray(log_step))
    sg = float(np.asarray(sigma))
    ns = int(np.asarray(num_steps))

    batch, dmodel = x.shape
    assert dmodel == D and batch % N_CORES == 0

    if ns <= 0:
        return x.copy()

    # Host-side parameter folding (O(D*K), trivial). qT/qmu/C use the
    # bf16-rounded W so they are consistent with the device projection.
    W = U / (S + 1e-6)[None, :]
    W16 = W.astype(ml_dtypes.bfloat16)
    Wq = W16.astype(np.float64)
    qT = tgt @ Wq
    qmu = mu @ Wq
    wt = qT - qmu
    Cc = float(wt @ wt)
    inv = 1.0 / (float(KSUB) * 2.0 * sg * sg * 1.0)  # TEMPERATURE = 1.0
    step = min(max(math.exp(ls), 1e-3), 1.0)
    h = step / ns

    neg_inv = -inv
    exp_bias = -inv * Cc
    neg_h = -h

    rows = batch // N_CORES
    nc = _get_program(rows, ns, neg_inv, exp_bias, neg_h)

    abr = np.zeros((128, 2), np.float32)
    abr[0:KSUB, 0] = 1.0
    abr[KSUB:128, 1] = (2.0 * wt).astype(np.float32)
    common = {
        "w": np.ascontiguousarray(W16),
        "tgt": np.ascontiguousarray(tgt.astype(ml_dtypes.bfloat16)[None, :]),
        "nqt": np.ascontiguousarray((-qT).astype(np.float32)[:, None]),
        "abr": abr,
    }
    in_maps = [
        {"x": x[i * rows : (i + 1) * rows], **common} for i in range(N_CORES)
    ]

    trace = bool(int(os.environ.get("GOF_TRACE", "0")))
    res = run_bass_kernel_spmd(nc, in_maps, list(range(N_CORES)), trace=trace)
    LAST_RESULT = res
    out = np.concatenate([res.results[i]["out"] for i in range(N_CORES)],
                         axis=0)
    return out


# revision 11
# speedup vs baseline: 1.3490x; 1.3490x over previous
"""Trainium2 Bass kernel for the GatedODEFlow problem.

Math: the reference iterates  a <- a + h*alpha(a) * (tgt - a)  where
alpha depends on a only through the low-rank projection (a - mu) @ U / S.
Since each step is a per-row convex blend toward the fixed vector tgt,
a_t = c_t * x + (1 - c_t) * tgt  for a per-row scalar c_t, and the
projection evolves affinely in c_t:

    proj_t = c_t * (x@W - tgt@W) + (tgt@W - mu@W)   with W = U / (S+1e-6)
    dist2_t = A * c_t^2 + B2 * c_t + C              (per-row A, B2; global C)
    alpha_t = exp(-dist2_t / (2*k*sigma^2))
    c_{t+1} = c_t * (1 - h * alpha_t),  c_0 = 1
    out = c_N * x + (1 - c_N) * tgt

So the device only needs ONE matmul q0 = x @ W per row plus a scalar
recurrence and a final fused blend: read x once, write out once
(memory-bound roofline).

The gate path (projection -> per-row scalars) runs in bf16 on the tensor
engine: it only determines the scalar alpha, whose sensitivity to input
rounding is tiny (dist^2 ~ hundreds, /128 in the exponent). The final
blend out = c*x + (1-c)*tgt reads the original fp32 x, so output
precision stays ~1e-4.

Sharding: data-parallel across 8 cores along the batch dim; small
parameters replicated (per the problem's sharding hint).
"""

import math
import os
from contextlib import ExitStack

import numpy as np
import ml_dtypes

import concourse.bass as bass
import concourse.mybir as mybir
import concourse.tile as tile
from concourse import bacc
from concourse.masks import make_identity
from concourse.bass_utils import run_bass_kernel_spmd

F32 = mybir.dt.float32
BF16 = mybir.dt.bfloat16
AF = mybir.ActivationFunctionType
OP = mybir.AluOpType

N_CORES = 8
D = 4096
KSUB = 64
SUB = 128            # rows per subblock (one partition tile)
SPM = 4              # subblocks per macroblock
MACRO = SUB * SPM    # 512 rows
DCH = 128            # d-chunk width for PE transposes
NDCH = D // DCH      # 32
XCH = 1024           # bf16 cast chunk width
NXCH = D // XCH      # 4
CCH = 512            # combine chunk width
NCCH = D // CCH      # 8

_PROGRAM_CACHE: dict = {}
LAST_RESULT = None


def _build_program(rows: int, num_steps: int, neg_inv: float, exp_bias: float,
                   neg_h: float):
    nmacro = rows // MACRO
    assert rows == nmacro * MACRO, f"rows {rows} not a multiple of {MACRO}"

    nc = bacc.Bacc("TRN2")
    x_d = nc.dram_tensor("x", [rows, D], F32, kind="ExternalInput")
    w_d = nc.dram_tensor("w", [D, KSUB], BF16, kind="ExternalInput")
    tgt_d = nc.dram_tensor("tgt", [1, D], BF16, kind="ExternalInput")
    nqt_d = nc.dram_tensor("nqt", [KSUB, 1], F32, kind="ExternalInput")
    abr_d = nc.dram_tensor("abr", [128, 2], F32, kind="ExternalInput")
    out_d = nc.dram_tensor("out", [rows, D], F32, kind="ExternalOutput")

    with ExitStack() as ctx:
        tc = ctx.enter_context(tile.TileContext(nc))
        singles = ctx.enter_context(tc.tile_pool(name="singles", bufs=1))
        xpool = ctx.enter_context(tc.tile_pool(name="xp", bufs=8))
        xtpool = ctx.enter_context(tc.tile_pool(name="xtp", bufs=3))
        stkpool = ctx.enter_context(tc.tile_pool(name="stkp", bufs=2))
        smpool = ctx.enter_context(tc.tile_pool(name="smp", bufs=2))
        ptr = ctx.enter_context(tc.tile_pool(name="ptr", bufs=2, space="PSUM"))
        pq = ctx.enter_context(tc.tile_pool(name="pq", bufs=1, space="PSUM"))
        pab = ctx.enter_context(tc.tile_pool(name="pab", bufs=1, space="PSUM"))
        pdt = ctx.enter_context(tc.tile_pool(name="pdt", bufs=1, space="PSUM"))
        pout = ctx.enter_context(tc.tile_pool(name="pout", bufs=3, space="PSUM"))

        ident32 = singles.tile([128, 128], F32)
        make_identity(nc, ident32)
        w_sb = singles.tile([128, NDCH, KSUB], BF16)
        nc.sync.dma_start(out=w_sb, in_=w_d[:, :].rearrange("(j p) k -> p j k", p=128))
        tgt_sb = singles.tile([1, D], BF16)
        nc.sync.dma_start(out=tgt_sb, in_=tgt_d[:, :])
        nqt_sb = singles.tile([KSUB, 1], F32)
        nc.sync.dma_start(out=nqt_sb, in_=nqt_d[:, :])
        abr_sb = singles.tile([128, 2], F32)
        nc.sync.dma_start(out=abr_sb, in_=abr_d[:, :])
        ebias_sb = singles.tile([128, 1], F32)
        nc.vector.memset(ebias_sb, exp_bias)

        for m in range(nmacro):
            r0 = m * MACRO
            # -------- load x tiles (natural layout, kept for the blend) ----
            xs = []
            for s in range(SPM):
                xin = xpool.tile([SUB, D], F32, tag="xin")
                nc.sync.dma_start(
                    out=xin, in_=x_d[r0 + s * SUB : r0 + (s + 1) * SUB, :])
                xs.append(xin)

            # -- projection q0T[k,b]: f32 PE-transpose, bf16 copy+matmul ----
            q0T = pq.tile([KSUB, MACRO], F32, tag="q0T")
            for j in range(NDCH):
                tp = ptr.tile([128, MACRO], F32, tag="tp")
                for s in range(SPM):
                    nc.tensor.transpose(
                        tp[:, s * SUB : (s + 1) * SUB],
                        xs[s][:, j * DCH : (j + 1) * DCH], ident32)
                xt = xtpool.tile([128, MACRO], BF16, tag="xt")
                nc.scalar.copy(xt, tp)
                nc.tensor.matmul(
                    q0T, w_sb[:, j, :], xt,
                    start=(j == 0), stop=(j == NDCH - 1))

            # -------- per-row A and B2 via tiny reductions -----------------
            # stk rows 0..63  = (q0T - qT)^2 ; rows 64..127 = (q0T - qT)
            stk = stkpool.tile([128, MACRO], F32, tag="stk")
            nc.scalar.activation(stk[0:KSUB, :], q0T, AF.Square,
                                 bias=nqt_sb, scale=1.0)
            nc.scalar.activation(stk[KSUB:128, :], q0T, AF.Identity,
                                 bias=nqt_sb, scale=1.0)
            ab = pab.tile([128, 2 * SPM], F32, tag="ab")
            for s in range(SPM):
                lhs = stk[:, s * SUB : (s + 1) * SUB]
                nc.tensor.matmul(ab[:, s : s + 1], lhs,
                                 abr_sb[:, 0:1],
                                 start=True, stop=True)
                nc.tensor.matmul(ab[:, SPM + s : SPM + s + 1], lhs,
                                 abr_sb[:, 1:2],
                                 start=True, stop=True)
            A = ab[:, 0:SPM]
            B2 = ab[:, SPM : 2 * SPM]

            # -------- scalar recurrence on c  [128, SPM] -------------------
            c = smpool.tile([128, SPM], F32, tag="c")
            nc.vector.memset(c, 1.0)
            t1 = smpool.tile([128, SPM], F32, tag="t1")
            alpha = smpool.tile([128, SPM], F32, tag="alpha")
            for _t in range(num_steps):
                nc.vector.tensor_tensor(t1, A, c, OP.mult)
                nc.vector.tensor_tensor(t1, t1, B2, OP.add)
                nc.vector.tensor_tensor(t1, t1, c, OP.mult)
                # alpha = exp(neg_inv * dist' + (-inv*C))
                nc.scalar.activation(alpha, t1, AF.Exp,
                                     bias=ebias_sb, scale=neg_inv)
                nc.vector.tensor_tensor(t1, alpha, c, OP.mult)
                # c = (t1 * -h) + c
                nc.vector.scalar_tensor_tensor(c, t1, neg_h, c, OP.mult, OP.add)

            # d = 1 - c, transposed into a free-dim row per subblock
            d_t = smpool.tile([128, SPM], F32, tag="d")
            nc.vector.tensor_scalar(d_t, c, -1.0, 1.0, OP.mult, OP.add)
            dT = pdt.tile([1, MACRO], F32, tag="dT")
            drows = []
            for s in range(SPM):
                nc.tensor.transpose(dT[:, s * SUB : (s + 1) * SUB],
                                    d_t[:, s : s + 1], ident32)
                dr = smpool.tile([1, SUB], BF16, tag=f"dr{s}")
                nc.vector.tensor_copy(dr, dT[:, s * SUB : (s + 1) * SUB])
                drows.append(dr)

            # -------- blend: out = c*x + (1-c)*tgt, in place over x --------
            for s in range(SPM):
                for h2 in range(NCCH):
                    op_ps = pout.tile([128, CCH], F32, tag="op")
                    nc.tensor.matmul(
                        op_ps, drows[s],
                        tgt_sb[:, h2 * CCH : (h2 + 1) * CCH],
                        start=True, stop=True)
                    xsl = xs[s][:, h2 * CCH : (h2 + 1) * CCH]
                    nc.vector.scalar_tensor_tensor(
                        xsl, xsl, c[:, s : s + 1], op_ps, OP.mult, OP.add)
                nc.sync.dma_start(
                    out=out_d[r0 + s * SUB : r0 + (s + 1) * SUB, :], in_=xs[s])

    if not nc.is_finalized():
        nc.finalize()
    return nc


def _get_program(rows, num_steps, neg_inv, exp_bias, neg_h):
    key = (rows, num_steps, neg_inv, exp_bias, neg_h)
    if key not in _PROGRAM_CACHE:
        _PROGRAM_CACHE[key] = _build_program(rows, num_steps, neg_inv,
                                             exp_bias, neg_h)
    return _PROGRAM_CACHE[key]


def kernel(x, manifold_mu, manifold_U, manifold_S, attractor_mu,
           log_step, sigma, num_steps):
    global LAST_RESULT
    x = np.ascontiguousarray(np.asarray(x, dtype=np.float32))
    mu = np.asarray(manifold_mu, dtype=np.float64)
    U = np.asarray(manifold_U, dtype=np.float64)
    S = np.asarray(manifold_S, dtype=np.float64)
    tgt = np.asarray(attractor_mu, dtype=np.float64)
    ls = float(np.asarray(log_step))
    sg = float(np.asarray(sigma))
    ns = int(np.asarray(num_steps))

    batch, dmodel = x.shape
    assert dmodel == D and batch % N_CORES == 0

    if ns <= 0:
        return x.copy()

    # Host-side parameter folding (O(D*K), trivial). qT/qmu/C use the
    # bf16-rounded W so they are consistent with the device projection.
    W = U / (S + 1e-6)[None, :]
    W16 = W.astype(ml_dtypes.bfloat16)
    Wq = W16.astype(np.float64)
    qT = tgt @ Wq
    qmu = mu @ Wq
    wt = qT - qmu
    Cc = float(wt @ wt)
    inv = 1.0 / (float(KSUB) * 2.0 * sg * sg * 1.0)  # TEMPERATURE = 1.0
    step = min(max(math.exp(ls), 1e-3), 1.0)
    h = step / ns

    neg_inv = -inv
    exp_bias = -inv * Cc
    neg_h = -h

    rows = batch // N_CORES
    nc = _get_program(rows, ns, neg_inv, exp_bias, neg_h)

    abr = np.zeros((128, 2), np.float32)
    abr[0:KSUB, 0] = 1.0
    abr[KSUB:128, 1] = (2.0 * wt).astype(np.float32)
    common = {
        "w": np.ascontiguousarray(W16),
        "tgt": np.ascontiguousarray(tgt.astype(ml_dtypes.bfloat16)[None, :]),
        "nqt": np.ascontiguousarray((-qT).astype(np.float32)[:, None]),
        "abr": abr,
    }
    in_maps = [
        {"x": x[i * rows : (i + 1) * rows], **common} for i in range(N_CORES)
    ]

    trace = bool(int(os.environ.get("GOF_TRACE", "0")))
    res = run_bass_kernel_spmd(nc, in_maps, list(range(N_CORES)), trace=trace)
    LAST_RESULT = res
    out = np.concatenate([res.results[i]["out"] for i in range(N_CORES)],
                         axis=0)
    return out


# revision 12
# speedup vs baseline: 1.3728x; 1.0176x over previous
"""Trainium2 Bass kernel for the GatedODEFlow problem.

Math: the reference iterates  a <- a + h*alpha(a) * (tgt - a)  where
alpha depends on a only through the low-rank projection (a - mu) @ U / S.
Since each step is a per-row convex blend toward the fixed vector tgt,
a_t = c_t * x + (1 - c_t) * tgt  for a per-row scalar c_t, and the
projection evolves affinely in c_t:

    proj_t = c_t * (x@W - tgt@W) + (tgt@W - mu@W)   with W = U / (S+1e-6)
    dist2_t = A * c_t^2 + B2 * c_t + C              (per-row A, B2; global C)
    alpha_t = exp(-dist2_t / (2*k*sigma^2))
    c_{t+1} = c_t * (1 - h * alpha_t),  c_0 = 1
    out = c_N * x + (1 - c_N) * tgt

So the device only needs ONE matmul q0 = x @ W per row plus a scalar
recurrence and a final fused blend: read x once, write out once
(memory-bound roofline).

The gate path (projection -> per-row scalars) runs in bf16 on the tensor
engine: it only determines the scalar alpha, whose sensitivity to input
rounding is tiny (dist^2 ~ hundreds, /128 in the exponent). The final
blend out = c*x + (1-c)*tgt reads the original fp32 x, so output
precision stays ~1e-4.

Sharding: data-parallel across 8 cores along the batch dim; small
parameters replicated (per the problem's sharding hint).
"""

import math
import os
from contextlib import ExitStack

import numpy as np
import ml_dtypes

import concourse.bass as bass
import concourse.mybir as mybir
import concourse.tile as tile
from concourse import bacc
from concourse.masks import make_identity
from concourse.bass_utils import run_bass_kernel_spmd

F32 = mybir.dt.float32
BF16 = mybir.dt.bfloat16
AF = mybir.ActivationFunctionType
OP = mybir.AluOpType

N_CORES = 8
D = 4096
KSUB = 64
SUB = 128            # rows per subblock (one partition tile)
SPM = 4              # subblocks per macroblock
MACRO = SUB * SPM    # 512 rows
DCH = 128            # d-chunk width for PE transposes
NDCH = D // DCH      # 32
XCH = 1024           # bf16 cast chunk width
NXCH = D // XCH      # 4
CCH = 512            # combine chunk width
NCCH = D // CCH      # 8

_PROGRAM_CACHE: dict = {}
LAST_RESULT = None


def _build_program(rows: int, num_steps: int, neg_inv: float, exp_bias: float,
                   neg_h: float):
    nmacro = rows // MACRO
    assert rows == nmacro * MACRO, f"rows {rows} not a multiple of {MACRO}"

    nc = bacc.Bacc("TRN2")
    x_d = nc.dram_tensor("x", [rows, D], F32, kind="ExternalInput")
    w_d = nc.dram_tensor("w", [D, KSUB], BF16, kind="ExternalInput")
    tgt_d = nc.dram_tensor("tgt", [1, D], BF16, kind="ExternalInput")
    nqt_d = nc.dram_tensor("nqt", [KSUB, 1], F32, kind="ExternalInput")
    abr_d = nc.dram_tensor("abr", [128, 2], F32, kind="ExternalInput")
    out_d = nc.dram_tensor("out", [rows, D], F32, kind="ExternalOutput")

    with ExitStack() as ctx:
        tc = ctx.enter_context(tile.TileContext(nc))
        singles = ctx.enter_context(tc.tile_pool(name="singles", bufs=1))
        xpool = ctx.enter_context(tc.tile_pool(name="xp", bufs=8))
        xtpool = ctx.enter_context(tc.tile_pool(name="xtp", bufs=3))
        stkpool = ctx.enter_context(tc.tile_pool(name="stkp", bufs=2))
        smpool = ctx.enter_context(tc.tile_pool(name="smp", bufs=2))
        ptr = ctx.enter_context(tc.tile_pool(name="ptr", bufs=2, space="PSUM"))
        pq = ctx.enter_context(tc.tile_pool(name="pq", bufs=1, space="PSUM"))
        pab = ctx.enter_context(tc.tile_pool(name="pab", bufs=1, space="PSUM"))
        pdt = ctx.enter_context(tc.tile_pool(name="pdt", bufs=1, space="PSUM"))
        pout = ctx.enter_context(tc.tile_pool(name="pout", bufs=3, space="PSUM"))

        ident32 = singles.tile([128, 128], F32)
        make_identity(nc, ident32)
        w_sb = singles.tile([128, NDCH, KSUB], BF16)
        nc.sync.dma_start(out=w_sb, in_=w_d[:, :].rearrange("(j p) k -> p j k", p=128))
        tgt_sb = singles.tile([1, D], BF16)
        nc.sync.dma_start(out=tgt_sb, in_=tgt_d[:, :])
        nqt_sb = singles.tile([KSUB, 1], F32)
        nc.sync.dma_start(out=nqt_sb, in_=nqt_d[:, :])
        abr_sb = singles.tile([128, 2], F32)
        nc.sync.dma_start(out=abr_sb, in_=abr_d[:, :])
        ebias_sb = singles.tile([128, 1], F32)
        nc.vector.memset(ebias_sb, exp_bias)

        def emit_front(m):
            """Loads + PE transposes + bf16 projection + extraction + A/B."""
            r0 = m * MACRO
            xs = []
            for s in range(SPM):
                xin = xpool.tile([SUB, D], F32, tag="xin")
                nc.sync.dma_start(
                    out=xin, in_=x_d[r0 + s * SUB : r0 + (s + 1) * SUB, :])
                xs.append(xin)

            q0T = pq.tile([KSUB, MACRO], F32, tag="q0T")
            for j in range(NDCH):
                tp = ptr.tile([128, MACRO], F32, tag="tp")
                for s in range(SPM):
                    nc.tensor.transpose(
                        tp[:, s * SUB : (s + 1) * SUB],
                        xs[s][:, j * DCH : (j + 1) * DCH], ident32)
                xt = xtpool.tile([128, MACRO], BF16, tag="xt")
                nc.scalar.copy(xt, tp)
                nc.tensor.matmul(
                    q0T, w_sb[:, j, :], xt,
                    start=(j == 0), stop=(j == NDCH - 1))

            # stk rows 0..63 = (q0T - qT)^2 ; rows 64..127 = (q0T - qT)
            stk = stkpool.tile([128, MACRO], F32, tag="stk")
            nc.scalar.activation(stk[0:KSUB, :], q0T, AF.Square,
                                 bias=nqt_sb, scale=1.0)
            nc.scalar.activation(stk[KSUB:128, :], q0T, AF.Identity,
                                 bias=nqt_sb, scale=1.0)
            ab = pab.tile([128, 2 * SPM], F32, tag="ab")
            for s in range(SPM):
                lhs = stk[:, s * SUB : (s + 1) * SUB]
                nc.tensor.matmul(ab[:, s : s + 1], lhs,
                                 abr_sb[:, 0:1], start=True, stop=True)
                nc.tensor.matmul(ab[:, SPM + s : SPM + s + 1], lhs,
                                 abr_sb[:, 1:2], start=True, stop=True)
            return {"xs": xs, "ab": ab, "r0": r0}

        def emit_iteration(st):
            """Per-row scalar recurrence (DVE + ACT exp) -> c, d."""
            ab = st["ab"]
            A = ab[:, 0:SPM]
            B2 = ab[:, SPM : 2 * SPM]
            c = smpool.tile([128, SPM], F32, tag="c")
            nc.vector.memset(c, 1.0)
            t1 = smpool.tile([128, SPM], F32, tag="t1")
            alpha = smpool.tile([128, SPM], F32, tag="alpha")
            for _t in range(num_steps):
                nc.vector.tensor_tensor(t1, A, c, OP.mult)
                nc.vector.tensor_tensor(t1, t1, B2, OP.add)
                nc.vector.tensor_tensor(t1, t1, c, OP.mult)
                nc.scalar.activation(alpha, t1, AF.Exp,
                                     bias=ebias_sb, scale=neg_inv)
                nc.vector.tensor_tensor(t1, alpha, c, OP.mult)
                nc.vector.scalar_tensor_tensor(c, t1, neg_h, c, OP.mult, OP.add)
            d_t = smpool.tile([128, SPM], F32, tag="d")
            nc.vector.tensor_scalar(d_t, c, -1.0, 1.0, OP.mult, OP.add)
            st["c"] = c
            st["d_t"] = d_t

        def emit_drows(st):
            """PE-transpose d into per-subblock rows (input ready by now)."""
            dT = pdt.tile([1, MACRO], F32, tag="dT")
            drows = []
            for s in range(SPM):
                nc.tensor.transpose(dT[:, s * SUB : (s + 1) * SUB],
                                    st["d_t"][:, s : s + 1], ident32)
                dr = smpool.tile([1, SUB], BF16, tag=f"dr{s}")
                nc.vector.tensor_copy(dr, dT[:, s * SUB : (s + 1) * SUB])
                drows.append(dr)
            st["drows"] = drows

        def emit_blend_store(st):
            """out = c*x + (1-c)*tgt in place over x, then store."""
            xs, c, drows, r0 = st["xs"], st["c"], st["drows"], st["r0"]
            for s in range(SPM):
                for h2 in range(NCCH):
                    op_ps = pout.tile([128, CCH], F32, tag="op")
                    nc.tensor.matmul(
                        op_ps, drows[s],
                        tgt_sb[:, h2 * CCH : (h2 + 1) * CCH],
                        start=True, stop=True)
                    xsl = xs[s][:, h2 * CCH : (h2 + 1) * CCH]
                    nc.vector.scalar_tensor_tensor(
                        xsl, xsl, c[:, s : s + 1], op_ps, OP.mult, OP.add)
                nc.sync.dma_start(
                    out=out_d[r0 + s * SUB : r0 + (s + 1) * SUB, :],
                    in_=xs[s])

        # Software-pipelined emission: macro m's back-half (d-transpose,
        # outer products, blend) is emitted after macro m+1's PE-heavy
        # front so the in-order PE stream never waits on the serial
        # DVE/ACT recurrence.
        prev = None
        for m in range(nmacro):
            st = emit_front(m)
            if prev is not None:
                emit_drows(prev)
            emit_iteration(st)
            if prev is not None:
                emit_blend_store(prev)
            prev = st
        emit_drows(prev)
        emit_blend_store(prev)

    if not nc.is_finalized():
        nc.finalize()
    return nc


def _get_program(rows, num_steps, neg_inv, exp_bias, neg_h):
    key = (rows, num_steps, neg_inv, exp_bias, neg_h)
    if key not in _PROGRAM_CACHE:
        _PROGRAM_CACHE[key] = _build_program(rows, num_steps, neg_inv,
                                             exp_bias, neg_h)
    return _PROGRAM_CACHE[key]


def kernel(x, manifold_mu, manifold_U, manifold_S, attractor_mu,
           log_step, sigma, num_steps):
    global LAST_RESULT
    x = np.ascontiguousarray(np.asarray(x, dtype=np.float32))
    mu = np.asarray(manifold_mu, dtype=np.float64)
    U = np.asarray(manifold_U, dtype=np.float64)
    S = np.asarray(manifold_S, dtype=np.float64)
    tgt = np.asarray(attractor_mu, dtype=np.float64)
    ls = float(np.asarray(log_step))
    sg = float(np.asarray(sigma))
    ns = int(np.asarray(num_steps))

    batch, dmodel = x.shape
    assert dmodel == D and batch % N_CORES == 0

    if ns <= 0:
        return x.copy()

    # Host-side parameter folding (O(D*K), trivial). qT/qmu/C use the
    # bf16-rounded W so they are consistent with the device projection.
    W = U / (S + 1e-6)[None, :]
    W16 = W.astype(ml_dtypes.bfloat16)
    Wq = W16.astype(np.float64)
    qT = tgt @ Wq
    qmu = mu @ Wq
    wt = qT - qmu
    Cc = float(wt @ wt)
    inv = 1.0 / (float(KSUB) * 2.0 * sg * sg * 1.0)  # TEMPERATURE = 1.0
    step = min(max(math.exp(ls), 1e-3), 1.0)
    h = step / ns

    neg_inv = -inv
    exp_bias = -inv * Cc
    neg_h = -h

    rows = batch // N_CORES
    nc = _get_program(rows, ns, neg_inv, exp_bias, neg_h)

    abr = np.zeros((128, 2), np.float32)
    abr[0:KSUB, 0] = 1.0
    abr[KSUB:128, 1] = (2.0 * wt).astype(np.float32)
    common = {
        "w": np.ascontiguousarray(W16),
        "tgt": np.ascontiguousarray(tgt.astype(ml_dtypes.bfloat16)[None, :]),
        "nqt": np.ascontiguousarray((-qT).astype(np.float32)[:, None]),
        "abr": abr,
    }
    in_maps = [
        {"x": x[i * rows : (i + 1) * rows], **common} for i in range(N_CORES)
    ]

    trace = bool(int(os.environ.get("GOF_TRACE", "0")))
    res = run_bass_kernel_spmd(nc, in_maps, list(range(N_CORES)), trace=trace)
    LAST_RESULT = res
    out = np.concatenate([res.results[i]["out"] for i in range(N_CORES)],
                         axis=0)
    return out
